# revision 7
# baseline (speedup 1.0000x reference)
"""ALiBi multi-head attention, tensor-parallel over heads on 8 TRN2 NeuronCores.

Sharding: 16 heads / 8 cores = 2 heads per core. Each core computes the QKV
projection for its 2 heads (1/sqrt(dh) folded into the q weights), full
attention for both batches, and a partial output projection through its head
slice of out_w. The host sums the 8 partial outputs (the TP all-reduce done as
the unshard step) and adds out_b.

v2 layout (vs v1): both heads' score matmuls are issued adjacently so the PE
packs them as row tiles (K=64 at rows 0/64); PV drops the ones-column and
packs the two heads as col tiles (M=64 at cols 0/64) accumulating into one
PSUM bank (a zero-weight dummy matmul clears the bank's has_written bits once
so the two interleaved accumulation chains never reset each other); softmax
denominators come from M=1 ones-matmuls col-packed 4-wide into one bank; the
reciprocal is broadcast across partitions with a K=1 PE matmul instead of a
DRAM round-trip. The kb loop is software-pipelined (PV trails scores by 2) so
the scalar engine's exp stream never starves the PE.
"""

import os
import sys

for _p in ("/opt/trn_rl_repo",):
    if _p not in sys.path and os.path.isdir(_p):
        sys.path.insert(0, _p)

import numpy as np
import ml_dtypes

B = 2
S = 2048
D = 1024
H = 16
DH = 64
M_SLOPE = 0.5
T = B * S
N_CORES = 8
HPC = H // N_CORES  # heads per core
EW = 2 * S - 128  # 3968: ALiBi sliding-window table width
SCALE = 1.0 / np.sqrt(DH)

_CACHE = {}
last_results = None  # test harness reads exec_time_ns off this


def _bf16(a):
    return np.ascontiguousarray(np.asarray(a, dtype=np.float32)).astype(
        ml_dtypes.bfloat16
    )


def _build(reps=1):
    import concourse.mybir as mybir
    import concourse.tile as tile
    from concourse import bacc
    from contextlib import ExitStack, nullcontext

    f32 = mybir.dt.float32
    bf = mybir.dt.bfloat16
    AF = mybir.ActivationFunctionType
    MULT = mybir.AluOpType.mult

    nc = bacc.Bacc("TRN2", target_bir_lowering=False, debug=False, num_devices=N_CORES)

    xT_d = nc.dram_tensor("xT", [D, T], bf, kind="ExternalInput").ap()
    wqk_d = nc.dram_tensor("wqkT", [D, 256], bf, kind="ExternalInput").ap()
    wv_d = nc.dram_tensor("wvT", [D, 128], bf, kind="ExternalInput").ap()
    qkb_d = nc.dram_tensor("qkb", [128, 2], f32, kind="ExternalInput").ap()
    ow_d = nc.dram_tensor("owT", [128, D], bf, kind="ExternalInput").ap()
    e_d = [
        nc.dram_tensor(f"e{h}", [128, EW], bf, kind="ExternalInput").ap()
        for h in range(HPC)
    ]
    out_d = nc.dram_tensor("out", [T, D], bf, kind="ExternalOutput").ap()

    NTC = T // 512  # 8 token chunks of 512
    NDC = D // 128  # 8 d_model chunks
    NKB = S // 128  # 16 key blocks per sequence
    NTB = T // 128  # 32 token blocks of 128

    # den-psum row for each (head, q-half)
    DNROW = {(0, 0): 0, (0, 1): 32, (1, 0): 64, (1, 1): 96}

    with tile.TileContext(nc) as tc, ExitStack() as ctx:
        const = ctx.enter_context(tc.tile_pool(name="const", bufs=1))
        xpool = ctx.enter_context(tc.tile_pool(name="xp", bufs=16))
        big = ctx.enter_context(tc.tile_pool(name="big", bufs=1))
        ptp = ctx.enter_context(tc.tile_pool(name="ptp", bufs=8))
        stg = ctx.enter_context(tc.tile_pool(name="stg", bufs=4))
        ps = ctx.enter_context(tc.tile_pool(name="ps", bufs=1, space="PSUM"))

        # ---- constants ----
        wqk_sb = const.tile([128, NDC, 256], bf, tag="wqk")
        nc.sync.dma_start(out=wqk_sb[:], in_=wqk_d.rearrange("(c p) r -> p c r", p=128))
        wv_sb = const.tile([128, NDC, 128], bf, tag="wv")
        nc.sync.dma_start(out=wv_sb[:], in_=wv_d.rearrange("(c p) r -> p c r", p=128))
        qkb_sb = const.tile([128, 2], f32, tag="qkb")
        nc.sync.dma_start(out=qkb_sb[:], in_=qkb_d[:, :])
        ones_c = const.tile([128, 64], bf, tag="ones")
        nc.vector.memset(ones_c[:], 1.0)
        zeros_c = const.tile([128, 128], bf, tag="zeros")
        nc.vector.memset(zeros_c[:], 0.0)

        # ---- persistent activation tiles ----
        qT = big.tile([128, T], bf, tag="qT")  # rows: h0 ch 0-63 | h1 ch 64-127
        kT = big.tile([128, T], bf, tag="kT")
        # token-major V: block tb holds [h0 v(64) | h1 v(64)]
        vb = big.tile([128, NTB * 128], bf, tag="vb")
        oT = [big.tile([128, S], bf, tag=f"oT{b}", name=f"oT{b}") for b in range(B)]

        loop_cm = tc.For_i(0, reps, 1) if reps > 1 else nullcontext()
        with loop_cm:
            # ---- QKV projection, streaming x ----
            for tci in range(NTC):
                xts = []
                for dc in range(NDC):
                    xt = xpool.tile([128, 512], bf, tag="xt")
                    nc.sync.dma_start(
                        out=xt[:],
                        in_=xT_d[dc * 128 : (dc + 1) * 128, tci * 512 : (tci + 1) * 512],
                    )
                    xts.append(xt)
                # V token-major: psv[tok, vdim] copied wholesale into vb
                for half in range(4):
                    tb = tci * 4 + half
                    psv = ps.tile([128, 128], f32, tag="pp", name="psv", bufs=1)
                    for dc in range(NDC):
                        nc.tensor.matmul(
                            out=psv[:],
                            lhsT=xts[dc][:, half * 128 : (half + 1) * 128],
                            rhs=wv_sb[:, dc, :],
                            start=(dc == 0),
                            stop=(dc == NDC - 1),
                        )
                    nc.scalar.copy(
                        out=vb[:, tb * 128 : (tb + 1) * 128], in_=psv[:]
                    )
                # Q^T/K^T channel-major into one 2-bank psum tile
                psqk = ps.tile([128, 1024], f32, tag="sc", name="psqk", bufs=2)
                for dc in range(NDC):
                    st = dc == 0
                    sp = dc == NDC - 1
                    nc.tensor.matmul(
                        out=psqk[:, 0:512],
                        lhsT=wqk_sb[:, dc, 0:128],
                        rhs=xts[dc][:],
                        start=st,
                        stop=sp,
                    )
                    nc.tensor.matmul(
                        out=psqk[:, 512:1024],
                        lhsT=wqk_sb[:, dc, 128:256],
                        rhs=xts[dc][:],
                        start=st,
                        stop=sp,
                    )
                cs = slice(tci * 512, (tci + 1) * 512)
                nc.vector.tensor_scalar_add(
                    out=qT[:, cs], in0=psqk[:, 0:512], scalar1=qkb_sb[:, 0:1]
                )
                nc.vector.tensor_scalar_add(
                    out=kT[:, cs], in0=psqk[:, 512:1024], scalar1=qkb_sb[:, 1:2]
                )

            ow_sb = const.tile([128, D], bf, tag="ow")
            nc.sync.dma_start(out=ow_sb[:], in_=ow_d[:, :])
            e_sb = []
            for h in range(HPC):
                e = const.tile([128, EW], bf, tag=f"e{h}", name=f"e{h}sb")
                nc.sync.dma_start(out=e[:], in_=e_d[h][:, :])
                e_sb.append(e)

            # ---- attention: query chunks of 1024, kb pipeline with PV lag 2 ----
            for b in range(B):
                for qc in range(S // 1024):
                    pv = [
                        ps.tile([128, 512], f32, tag="pv", name=f"pv{half}", bufs=2)
                        for half in range(2)
                    ]
                    dn = ps.tile([128, 512], f32, tag="dn", name="dn", bufs=1)
                    # zero-weight dummies: clear has_written for the whole
                    # bank exactly once so the interleaved accumulation
                    # chains below can all run start=False
                    for t_ in (pv[0], pv[1], dn):
                        nc.tensor.matmul(
                            out=t_[:],
                            lhsT=zeros_c[:],
                            rhs=qT[:, 0:512],
                            start=True,
                            stop=False,
                            skip_group_check=True,
                        )
                    pts = {}
                    for kb in range(NKB + 2):
                        if kb < NKB:
                            ks = slice(b * S + kb * 128, b * S + kb * 128 + 128)
                            sc = [
                                ps.tile([128, 1024], f32, tag="sc", name=f"sc{h}", bufs=2)
                                for h in range(HPC)
                            ]
                            for half in range(2):
                                qs = slice(
                                    b * S + qc * 1024 + half * 512,
                                    b * S + qc * 1024 + half * 512 + 512,
                                )
                                for h in range(HPC):
                                    hs = slice(h * 64, h * 64 + 64)
                                    nc.tensor.matmul(
                                        out=sc[h][:, half * 512 : half * 512 + 512],
                                        lhsT=kT[hs, ks],
                                        rhs=qT[hs, qs],
                                        start=True,
                                        stop=True,
                                    )
                            for h in range(HPC):
                                pt = ptp.tile([128, 1024], bf, tag="pt")
                                nc.scalar.activation(out=pt[:], in_=sc[h][:], func=AF.Exp)
                                c0 = qc * 1024 - kb * 128 + (S - 128)
                                nc.vector.tensor_tensor(
                                    out=pt[:],
                                    in0=pt[:],
                                    in1=e_sb[h][:, c0 : c0 + 1024],
                                    op=MULT,
                                )
                                pts[(h, kb)] = pt
                        if kb >= 2:
                            kbp = kb - 2
                            kbg = b * NKB + kbp
                            sp = kbp == NKB - 1
                            for half in range(2):
                                hsl = slice(half * 512, half * 512 + 512)
                                for h in range(HPC):
                                    nc.tensor.matmul(
                                        out=pv[half][h * 64 : h * 64 + 64, :],
                                        lhsT=vb[:, kbg * 128 + h * 64 : kbg * 128 + h * 64 + 64],
                                        rhs=pts[(h, kbp)][:, hsl],
                                        start=False,
                                        stop=sp,
                                        skip_group_check=True,
                                    )
                            for half in range(2):
                                hsl = slice(half * 512, half * 512 + 512)
                                for h in range(HPC):
                                    r = DNROW[(h, half)]
                                    nc.tensor.matmul(
                                        out=dn[r : r + 1, :],
                                        lhsT=ones_c[:, 0:1],
                                        rhs=pts[(h, kbp)][:, hsl],
                                        start=False,
                                        stop=sp,
                                        skip_group_check=True,
                                        tile_position=(0, r),
                                    )
                    # ---- normalize: rcp + PE broadcast + multiply ----
                    rcp = stg.tile([128, 512], bf, tag="rcp")
                    with nc.allow_low_precision(reason="bf16 softmax reciprocal"):
                        for h in range(HPC):
                            for half in range(2):
                                r = DNROW[(h, half)]
                                nc.vector.reciprocal(
                                    out=rcp[r : r + 1, :], in_=dn[r : r + 1, :]
                                )
                    for half in range(2):
                        bc = ps.tile([128, 512], f32, tag="pp", name="bc", bufs=1)
                        for h in range(HPC):
                            r = DNROW[(h, half)]
                            nc.tensor.matmul(
                                out=bc[h * 64 : h * 64 + 64, :],
                                lhsT=ones_c[r : r + 1, 0:64],
                                rhs=rcp[r : r + 1, :],
                                start=True,
                                stop=True,
                                tile_position=(r, h * 64),
                            )
                        bcs = stg.tile([128, 512], bf, tag="bcs")
                        nc.vector.tensor_copy(out=bcs[:], in_=bc[:])
                        qoff = qc * 1024 + half * 512
                        nc.vector.tensor_tensor(
                            out=oT[b][:, qoff : qoff + 512],
                            in0=pv[half][:],
                            in1=bcs[:],
                            op=MULT,
                        )
                    # ---- partial output projection for this (b, qc) ----
                    for tb in range(qc * 8, qc * 8 + 8):
                        for nf in range(D // 512):
                            psp = ps.tile([128, 512], f32, tag="pp", name="psp", bufs=1)
                            nc.tensor.matmul(
                                out=psp[:],
                                lhsT=oT[b][:, tb * 128 : (tb + 1) * 128],
                                rhs=ow_sb[:, nf * 512 : (nf + 1) * 512],
                                start=True,
                                stop=True,
                            )
                            so = stg.tile([128, 512], bf, tag="so")
                            nc.vector.tensor_copy(out=so[:], in_=psp[:])
                            out_eng = (nc.sync, nc.scalar)[(tb * 2 + nf) % 2]
                            out_eng.dma_start(
                                out=out_d[
                                    b * S + tb * 128 : b * S + (tb + 1) * 128,
                                    nf * 512 : (nf + 1) * 512,
                                ],
                                in_=so[:],
                            )

    return nc


def _get_compiled():
    if "nc" not in _CACHE:
        nc = _build()
        nc.compile()
        _CACHE["nc"] = nc
    return _CACHE["nc"]


def _make_in_maps(x, qkv_w, qkv_b, out_w):
    x = np.asarray(x, dtype=np.float32)
    qkv_w = np.asarray(qkv_w, dtype=np.float32)
    qkv_b = np.asarray(qkv_b, dtype=np.float32)
    out_w = np.asarray(out_w, dtype=np.float32)
    xT = _bf16(x.reshape(T, D).T)
    p = np.arange(128, dtype=np.float64)[:, None]
    c = np.arange(EW, dtype=np.float64)[None, :]
    absd = np.abs(p + (S - 128.0) - c)
    in_maps = []
    for core in range(N_CORES):
        h0 = core * HPC
        # reference packs qkv_w rows per head: [h*192, h*192+192) = q|k|v
        wq, wk, wv, qb, kbi = [], [], [], [], []
        for h in (h0, h0 + 1):
            base = h * 3 * DH
            wq.append(qkv_w[base : base + DH, :] * SCALE)
            wk.append(qkv_w[base + DH : base + 2 * DH, :])
            wv.append(qkv_w[base + 2 * DH : base + 3 * DH, :])
            qb.append(qkv_b[base : base + DH] * SCALE)
            kbi.append(qkv_b[base + DH : base + 2 * DH])
        wqkT = _bf16(np.concatenate(wq + wk, axis=0).T)  # (D, 256)
        wvT = _bf16(np.concatenate(wv, axis=0).T)  # (D, 128)
        qkb = np.ascontiguousarray(
            np.stack([np.concatenate(qb), np.concatenate(kbi)], axis=1)
        ).astype(np.float32)
        owT = _bf16(out_w[:, h0 * DH : h0 * DH + 128].T)  # (128, D)
        m = {
            "xT": xT,
            "wqkT": wqkT,
            "wvT": wvT,
            "qkb": qkb,
            "owT": owT,
        }
        for h in range(HPC):
            slope = float(M_SLOPE ** (h0 + h))
            m[f"e{h}"] = np.exp(-slope * absd).astype(ml_dtypes.bfloat16)
        in_maps.append(m)
    return in_maps


def kernel(x, qkv_w, qkv_b, out_w, out_b):
    global last_results
    from concourse.bass_utils import run_bass_kernel_spmd

    nc = _get_compiled()
    in_maps = _make_in_maps(x, qkv_w, qkv_b, out_w)
    res = run_bass_kernel_spmd(
        nc,
        in_maps,
        core_ids=list(range(N_CORES)),
        trace=bool(os.environ.get("BASS_TRACE")),
    )
    last_results = res
    acc = np.zeros((T, D), dtype=np.float64)
    for c in range(N_CORES):
        acc += res.results[c]["out"].astype(np.float64)
    # v-bias folds out of the softmax average exactly: rows of P sum to 1, so
    # O = P(V + 1 vb^T)/denom = O_nobias + vb^T; project it on the host.
    qkv_b = np.asarray(qkv_b, dtype=np.float64)
    vb_full = np.concatenate(
        [qkv_b[h * 3 * DH + 2 * DH : h * 3 * DH + 3 * DH] for h in range(H)]
    )
    out = (
        acc
        + np.asarray(out_b, dtype=np.float64)[None, :]
        + (vb_full @ np.asarray(out_w, dtype=np.float64).T)[None, :]
    )
    return out.reshape(B, S, D).astype(np.float32)
